# revision 26
# baseline (speedup 1.0000x reference)
"""Trainium2 Bass kernel for nn_LowRankProjection: y = (spikes @ V) @ U.T.

Strategy (data-parallel over batch, 8 cores), fp16 wire format:
  - Host pre-layouts (all fp16 — harness gate is rel_err < 2e-2, fp16
    costs ~1e-3, and halving the bytes halves the HBM-bound runtime):
      sP  = spikes shard packed to the exact SBUF tile layout
            [SB*NT*128, KPER*BSB] so every input DMA is one fully
            contiguous 1 MiB transfer.
      Vd  = V rearranged to [128, KC*R] so lhsT chunks are slices.
      Ut  = U.T [R, N_POST]; replicated on-device to 4 partition strips.
      Rm  = 4x stacked I_32 [128, R] (strip-reduction matmul weight).
  - Device, per core, PIPELINED over 4 batch sub-blocks of 128 rows so
    the input and output streams overlap on the shared SDMA engines.
    Each HWDGE ring drains one DMA at a time (~260 GB/s), so both
    streams are split across two queues (input: sync ring + SWDGE;
    stores: scalar ring + SWDGE) to reach the 358 GB/s HBM ceiling.
    All SWDGE input emissions are hoisted ahead of the store emissions
    (the Q7 emits in program order, and a store emission waits on its
    o_tile, which would serialize input behind the expand pipeline).
    Per sub-block:
      project: 4-way col-group packed accumulation over 128 k-chunks:
               z4[32g+r, b] += V_k.T @ sT_k for k % 4 == g (tile_position)
      reduce:  zT = Rm.T @ z4 (one matmul contracts the 4 strips),
               replicated to 4 partition strips for row-group packing
      expand:  row-group packed matmuls into 2-bank PSUM regions;
               PSUM->SBUF casts to fp16 alternate vector/scalar engines
               (the only two engines with PSUM ports).
  - HBM per core: 16 MiB in + 16 MiB out + ~2 MiB weights ~= 95 us
    roofline at 358 GB/s; measured ~105-110 us incl. startup/drain.
"""

import numpy as np

import concourse.bacc as bacc
import concourse.mybir as mybir
import concourse.tile as tile
from concourse.bass_utils import run_bass_kernel_spmd

B, N_PRE, N_POST, R = 4096, 16384, 16384, 32
N_CORES = 8
BSH = B // N_CORES  # 512 batch rows per core
P = 128
KC = N_PRE // P  # 128 contraction chunks
F16 = mybir.dt.float16
F32 = mybir.dt.float32

SB = 4  # pipelined batch sub-blocks per core
BSB = BSH // SB  # 128 batch rows per sub-block
KPER = 64  # k-chunks per input DMA tile (2 MiB fp16)
NT = KC // KPER  # 2 input tiles per sub-block
OW = 8192  # output tile width (2 MiB fp16 stores)


def _body(tc, y, sP, vd, ut, rm):
    nc = tc.nc
    with (
        tc.tile_pool(name="w", bufs=1) as wpool,
        tc.tile_pool(name="s", bufs=4) as spool,
        tc.tile_pool(name="o", bufs=6) as opool,
        tc.tile_pool(name="z", bufs=2) as zpool,
        tc.tile_pool(name="zps", bufs=1, space="PSUM") as zpspool,
        tc.tile_pool(name="yps", bufs=3, space="PSUM") as ypspool,
    ):
        # Weights at the head of the scalar HWDGE ring (idle until the
        # first store ~30us in): they drain concurrently with the first
        # spike tile on the sync ring, so the first project matmul isn't
        # serialized behind them. On the slower SWDGE path they'd gate
        # the whole PE stream ~20us.
        v_sb = wpool.tile([P, KC * R], F16)
        nc.scalar.dma_start(v_sb[:], vd[:])
        rm_sb = wpool.tile([P, R], F16)
        nc.scalar.dma_start(rm_sb[:], rm[:])
        # Ut is COLUMN-PARTITIONED across the 4 row-group strips (strip g
        # holds only the n-chunks with chunk%4 == g), so no on-device
        # replication is needed at all — one 1 MiB load.
        utp = wpool.tile([P, N_POST // 4], F16)
        nc.scalar.dma_start(utp[:], ut[:])

        # ALL input DMAs are emitted up front, alternating between the
        # sync HWDGE ring and the gpsimd SWDGE queue (each ring drains
        # one DMA at a time at ~260 GB/s; two queues reach the 358 GB/s
        # HBM ceiling). Hoisting matters for the SWDGE queue: the Q7
        # emits descriptors in program order, so an input emission
        # placed after a store emission would wait on that store's
        # o_tile production, serializing the input stream behind the
        # expand pipeline.
        s_tiles = []
        for idx in range(SB * NT):
            s_tile = spool.tile([P, KPER * BSB], F16, name="s_tile", tag="s_tile")
            eng = nc.sync if idx % 2 == 0 else nc.gpsimd
            eng.dma_start(s_tile[:], sP[idx * P : (idx + 1) * P, :])
            s_tiles.append(s_tile)

        cp = 0
        for sb in range(SB):
            # --- project: z4 [128, BSB] = 4 col-group partial sums ---
            z4ps = zpspool.tile([P, BSB], F32, tag="z4")
            for t in range(NT):
                s_tile = s_tiles[sb * NT + t]
                for j in range(KPER):
                    k = t * KPER + j
                    g = k % 4
                    nc.tensor.matmul(
                        z4ps[g * R : (g + 1) * R, :],
                        v_sb[:, k * R : (k + 1) * R],
                        s_tile[:, j * BSB : (j + 1) * BSB],
                        start=(k < 4),
                        stop=(k >= KC - 4),
                        tile_position=(0, g * R),
                        # 4 interleaved per-strip groups share one bank;
                        # CoreSim's zero-region tracker is bank-coarse but
                        # HW has_written is per partition row.
                        skip_group_check=True,
                    )

            # --- reduce strips, replicate zT to 4 partition strips ---
            z4_sb = zpool.tile([P, BSB], F16, tag="z4sb")
            nc.vector.tensor_copy(z4_sb[:], z4ps[:])
            zps2 = zpspool.tile([R, BSB], F32, tag="zred")
            nc.tensor.matmul(zps2[:], rm_sb[:], z4_sb[:], start=True, stop=True)
            zt4 = zpool.tile([P, BSB], F16, tag="zt4")
            for g in range(4):
                if g % 2 == 0:
                    nc.vector.tensor_copy(zt4[g * R : (g + 1) * R, :], zps2[:])
                else:
                    nc.scalar.copy(zt4[g * R : (g + 1) * R, :], zps2[:])

            # --- expand: y[sb block, :] = zT.T @ Ut, row-group packed.
            # Group g computes the n-chunks with chunk%4 == g from its
            # own column-partitioned slice of utp.
            for grp in range(N_POST // OW):
                o_tile = opool.tile([P, OW], F16)
                for h in range(OW // 1024):
                    yp = ypspool.tile([P, 1024], F32)
                    for u in range(2):
                        g = (h % 2) * 2 + u
                        c = grp * (OW // 2048) + h // 2
                        nc.tensor.matmul(
                            yp[:, u * 512 : (u + 1) * 512],
                            zt4[g * R : (g + 1) * R, :],
                            utp[g * R : (g + 1) * R, c * 512 : (c + 1) * 512],
                            start=True,
                            stop=True,
                            tile_position=(g * R, 0),
                        )
                    dst = o_tile[:, h * 1024 : (h + 1) * 1024]
                    # Split PSUM->SBUF casts across the two engines
                    # with PSUM ports.
                    if cp % 2 == 0:
                        nc.vector.tensor_copy(dst, yp[:])
                    else:
                        nc.scalar.copy(dst, yp[:])
                    cp += 1
                # Alternate stores between the scalar HWDGE ring and the
                # gpsimd SWDGE queue for the same ring-cap reason. The
                # last sub-block's first store rides the sync ring, which
                # is idle once the input stream finishes.
                if sb == SB - 1 and grp == 0:
                    oeng = nc.sync
                elif (sb * (N_POST // OW) + grp) % 2 == 0:
                    oeng = nc.scalar
                else:
                    oeng = nc.gpsimd
                oeng.dma_start(
                    y[sb * P : (sb + 1) * P, grp * OW : (grp + 1) * OW],
                    o_tile[:],
                )


_NC_CACHE = None


def _build():
    global _NC_CACHE
    if _NC_CACHE is None:
        nc = bacc.Bacc(
            "TRN2", target_bir_lowering=False, debug=False, num_devices=N_CORES
        )
        sP = nc.dram_tensor(
            "sP", [SB * NT * P, KPER * BSB], F16, kind="ExternalInput"
        ).ap()
        vd = nc.dram_tensor("Vd", [P, KC * R], F16, kind="ExternalInput").ap()
        ut = nc.dram_tensor("Ut", [P, N_POST // 4], F16, kind="ExternalInput").ap()
        rm = nc.dram_tensor("Rm", [P, R], F16, kind="ExternalInput").ap()
        y = nc.dram_tensor("y", [BSH, N_POST], F16, kind="ExternalOutput").ap()
        with tile.TileContext(nc) as tc:
            _body(tc, y, sP, vd, ut, rm)
        nc.compile()
        _NC_CACHE = nc
    return _NC_CACHE


def _prep_inputs(spikes, U, V):
    spikes = np.asarray(spikes, dtype=np.float32)
    vd = np.ascontiguousarray(
        np.asarray(V, dtype=np.float32)
        .reshape(KC, P, R)
        .transpose(1, 0, 2)
        .reshape(P, KC * R)
        .astype(np.float16)
    )
    # Column-partitioned Ut: utp[32g+r, c*512+s] = U.T[r, c*2048+g*512+s],
    # so row-group strip g holds exactly the n-chunks it computes.
    ut = np.ascontiguousarray(
        np.asarray(U, dtype=np.float32)
        .T.astype(np.float16)
        .reshape(R, N_POST // 2048, 4, 512)
        .transpose(2, 0, 1, 3)
        .reshape(P, N_POST // 4)
    )
    rm = np.tile(np.eye(R, dtype=np.float16), (P // R, 1))
    in_maps = []
    for c in range(N_CORES):
        # [N_PRE, BSH] shard transpose (cache-friendly per-core blocks),
        # then pack to the SBUF tile layout [sb, t, p, j, b] so each
        # input DMA is one fully contiguous 1 MiB block.
        xt = spikes[c * BSH : (c + 1) * BSH].T.astype(np.float16)
        sp = np.ascontiguousarray(
            xt.reshape(NT, KPER, P, SB, BSB).transpose(3, 0, 2, 1, 4)
        ).reshape(SB * NT * P, KPER * BSB)
        in_maps.append({"sP": sp, "Vd": vd, "Ut": ut, "Rm": rm})
    return in_maps


def _run(spikes, U, V, **run_kwargs):
    nc = _build()
    in_maps = _prep_inputs(spikes, U, V)
    res = run_bass_kernel_spmd(nc, in_maps, list(range(N_CORES)), **run_kwargs)
    y = np.concatenate([res.results[c]["y"] for c in range(N_CORES)], axis=0).astype(
        np.float32
    )
    return y, res


def kernel(spikes, U, V, mask_row_ptr=None, mask_col_idx=None, mask_values=None):
    y, _ = _run(spikes, U, V)
    return y


# revision 28
# speedup vs baseline: 1.3033x; 1.3033x over previous
"""Trainium2 Bass kernel for nn_LowRankProjection: y = (spikes @ V) @ U.T.

Strategy (data-parallel over batch, 8 cores), fp16 wire format:
  - Host pre-layouts (all fp16 — harness gate is rel_err < 2e-2, fp16
    costs ~1e-3, and halving the bytes halves the HBM-bound runtime):
      sP  = spikes shard packed to the exact SBUF tile layout
            [SB*NT*128, KPER*BSB] so every input DMA is one fully
            contiguous 1 MiB transfer.
      Vd  = V rearranged to [128, KC*R] so lhsT chunks are slices.
      Ut  = U.T [R, N_POST]; replicated on-device to 4 partition strips.
      Rm  = 4x stacked I_32 [128, R] (strip-reduction matmul weight).
  - Device, per core, PIPELINED over 4 batch sub-blocks of 128 rows so
    the input and output streams overlap on the shared SDMA engines.
    Each HWDGE ring drains one DMA at a time (~260 GB/s), so both
    streams are split across two queues (input: sync ring + SWDGE;
    stores: scalar ring + SWDGE) to reach the 358 GB/s HBM ceiling.
    All SWDGE input emissions are hoisted ahead of the store emissions
    (the Q7 emits in program order, and a store emission waits on its
    o_tile, which would serialize input behind the expand pipeline).
    Per sub-block:
      project: 4-way col-group packed accumulation over 128 k-chunks:
               z4[32g+r, b] += V_k.T @ sT_k for k % 4 == g (tile_position)
      reduce:  zT = Rm.T @ z4 (one matmul contracts the 4 strips),
               replicated to 4 partition strips for row-group packing
      expand:  row-group packed matmuls into 2-bank PSUM regions;
               PSUM->SBUF casts to fp16 alternate vector/scalar engines
               (the only two engines with PSUM ports).
  - HBM per core: 16 MiB in + 16 MiB out + ~2 MiB weights ~= 95 us
    roofline at 358 GB/s; measured ~105-110 us incl. startup/drain.
"""

import numpy as np

import concourse.bacc as bacc
import concourse.mybir as mybir
import concourse.tile as tile
from concourse.bass_utils import run_bass_kernel_spmd

B, N_PRE, N_POST, R = 4096, 16384, 16384, 32
N_CORES = 8
BSH = B // N_CORES  # 512 batch rows per core
P = 128
KC = N_PRE // P  # 128 contraction chunks
F16 = mybir.dt.float16
F32 = mybir.dt.float32

SB = 4  # pipelined batch sub-blocks per core
BSB = BSH // SB  # 128 batch rows per sub-block
KPER = 64  # k-chunks per input DMA tile (2 MiB fp16)
NT = KC // KPER  # 2 input tiles per sub-block
OW = 8192  # output tile width (2 MiB fp16 stores)


def _body(tc, y, sP, vd, ut, rm):
    nc = tc.nc
    with (
        tc.tile_pool(name="w", bufs=1) as wpool,
        tc.tile_pool(name="s", bufs=4) as spool,
        tc.tile_pool(name="o", bufs=6) as opool,
        tc.tile_pool(name="z", bufs=2) as zpool,
        tc.tile_pool(name="zps", bufs=1, space="PSUM") as zpspool,
        tc.tile_pool(name="yps", bufs=3, space="PSUM") as ypspool,
    ):
        # V + reduction weights at the head of the sync HWDGE ring: they
        # drain in ~5us and the first project matmuls need them; on the
        # slower SWDGE path they'd gate the whole PE stream ~20us.
        v_sb = wpool.tile([P, KC * R], F16)
        nc.sync.dma_start(v_sb[:], vd[:])
        rm_sb = wpool.tile([P, R], F16)
        nc.sync.dma_start(rm_sb[:], rm[:])
        # Ut is COLUMN-PARTITIONED across the 4 row-group strips (strip g
        # holds only the n-chunks with chunk%4 == g), so no on-device
        # replication is needed at all — one 1 MiB load on the scalar
        # ring (idle until the first store ~30us in).
        utp = wpool.tile([P, N_POST // 4], F16)
        nc.scalar.dma_start(utp[:], ut[:])

        # ALL input DMAs are emitted up front, alternating between the
        # sync HWDGE ring and the gpsimd SWDGE queue (each ring drains
        # one DMA at a time at ~260 GB/s; two queues reach the 358 GB/s
        # HBM ceiling). Hoisting matters for the SWDGE queue: the Q7
        # emits descriptors in program order, so an input emission
        # placed after a store emission would wait on that store's
        # o_tile production, serializing the input stream behind the
        # expand pipeline.
        s_tiles = []
        for idx in range(SB * NT):
            s_tile = spool.tile([P, KPER * BSB], F16, name="s_tile", tag="s_tile")
            eng = nc.sync if idx % 2 == 0 else nc.gpsimd
            eng.dma_start(s_tile[:], sP[idx * P : (idx + 1) * P, :])
            s_tiles.append(s_tile)

        cp = 0
        for sb in range(SB):
            # --- project: z4 [128, BSB] = 4 col-group partial sums ---
            z4ps = zpspool.tile([P, BSB], F32, tag="z4")
            for t in range(NT):
                s_tile = s_tiles[sb * NT + t]
                for j in range(KPER):
                    k = t * KPER + j
                    g = k % 4
                    nc.tensor.matmul(
                        z4ps[g * R : (g + 1) * R, :],
                        v_sb[:, k * R : (k + 1) * R],
                        s_tile[:, j * BSB : (j + 1) * BSB],
                        start=(k < 4),
                        stop=(k >= KC - 4),
                        tile_position=(0, g * R),
                        # 4 interleaved per-strip groups share one bank;
                        # CoreSim's zero-region tracker is bank-coarse but
                        # HW has_written is per partition row.
                        skip_group_check=True,
                    )

            # --- reduce strips, replicate zT to 4 partition strips ---
            z4_sb = zpool.tile([P, BSB], F16, tag="z4sb")
            nc.vector.tensor_copy(z4_sb[:], z4ps[:])
            zps2 = zpspool.tile([R, BSB], F32, tag="zred")
            nc.tensor.matmul(zps2[:], rm_sb[:], z4_sb[:], start=True, stop=True)
            zt4 = zpool.tile([P, BSB], F16, tag="zt4")
            for g in range(4):
                nc.vector.tensor_copy(zt4[g * R : (g + 1) * R, :], zps2[:])

            # --- expand: y[sb block, :] = zT.T @ Ut, row-group packed.
            # Group g computes the n-chunks with chunk%4 == g from its
            # own column-partitioned slice of utp.
            for grp in range(N_POST // OW):
                o_tile = opool.tile([P, OW], F16)
                for h in range(OW // 1024):
                    yp = ypspool.tile([P, 1024], F32)
                    for u in range(2):
                        g = (h % 2) * 2 + u
                        c = grp * (OW // 2048) + h // 2
                        nc.tensor.matmul(
                            yp[:, u * 512 : (u + 1) * 512],
                            zt4[g * R : (g + 1) * R, :],
                            utp[g * R : (g + 1) * R, c * 512 : (c + 1) * 512],
                            start=True,
                            stop=True,
                            tile_position=(g * R, 0),
                        )
                    dst = o_tile[:, h * 1024 : (h + 1) * 1024]
                    # Split PSUM->SBUF casts across the two engines
                    # with PSUM ports.
                    if cp % 2 == 0:
                        nc.vector.tensor_copy(dst, yp[:])
                    else:
                        nc.scalar.copy(dst, yp[:])
                    cp += 1
                # Alternate stores between the scalar HWDGE ring and the
                # gpsimd SWDGE queue for the same ring-cap reason. The
                # last sub-block's first store rides the sync ring, which
                # is idle once the input stream finishes.
                if sb == SB - 1 and grp == 0:
                    oeng = nc.sync
                elif (sb * (N_POST // OW) + grp) % 2 == 0:
                    oeng = nc.scalar
                else:
                    oeng = nc.gpsimd
                oeng.dma_start(
                    y[sb * P : (sb + 1) * P, grp * OW : (grp + 1) * OW],
                    o_tile[:],
                )


_NC_CACHE = None


def _build():
    global _NC_CACHE
    if _NC_CACHE is None:
        nc = bacc.Bacc(
            "TRN2", target_bir_lowering=False, debug=False, num_devices=N_CORES
        )
        sP = nc.dram_tensor(
            "sP", [SB * NT * P, KPER * BSB], F16, kind="ExternalInput"
        ).ap()
        vd = nc.dram_tensor("Vd", [P, KC * R], F16, kind="ExternalInput").ap()
        ut = nc.dram_tensor("Ut", [P, N_POST // 4], F16, kind="ExternalInput").ap()
        rm = nc.dram_tensor("Rm", [P, R], F16, kind="ExternalInput").ap()
        y = nc.dram_tensor("y", [BSH, N_POST], F16, kind="ExternalOutput").ap()
        with tile.TileContext(nc) as tc:
            _body(tc, y, sP, vd, ut, rm)
        nc.compile()
        _NC_CACHE = nc
    return _NC_CACHE


def _prep_inputs(spikes, U, V):
    spikes = np.asarray(spikes, dtype=np.float32)
    vd = np.ascontiguousarray(
        np.asarray(V, dtype=np.float32)
        .reshape(KC, P, R)
        .transpose(1, 0, 2)
        .reshape(P, KC * R)
        .astype(np.float16)
    )
    # Column-partitioned Ut: utp[32g+r, c*512+s] = U.T[r, c*2048+g*512+s],
    # so row-group strip g holds exactly the n-chunks it computes.
    ut = np.ascontiguousarray(
        np.asarray(U, dtype=np.float32)
        .T.astype(np.float16)
        .reshape(R, N_POST // 2048, 4, 512)
        .transpose(2, 0, 1, 3)
        .reshape(P, N_POST // 4)
    )
    rm = np.tile(np.eye(R, dtype=np.float16), (P // R, 1))
    in_maps = []
    for c in range(N_CORES):
        # [N_PRE, BSH] shard transpose (cache-friendly per-core blocks),
        # then pack to the SBUF tile layout [sb, t, p, j, b] so each
        # input DMA is one fully contiguous 1 MiB block.
        xt = spikes[c * BSH : (c + 1) * BSH].T.astype(np.float16)
        sp = np.ascontiguousarray(
            xt.reshape(NT, KPER, P, SB, BSB).transpose(3, 0, 2, 1, 4)
        ).reshape(SB * NT * P, KPER * BSB)
        in_maps.append({"sP": sp, "Vd": vd, "Ut": ut, "Rm": rm})
    return in_maps


def _run(spikes, U, V, **run_kwargs):
    nc = _build()
    in_maps = _prep_inputs(spikes, U, V)
    res = run_bass_kernel_spmd(nc, in_maps, list(range(N_CORES)), **run_kwargs)
    y = np.concatenate([res.results[c]["y"] for c in range(N_CORES)], axis=0).astype(
        np.float32
    )
    return y, res


def kernel(spikes, U, V, mask_row_ptr=None, mask_col_idx=None, mask_values=None):
    y, _ = _run(spikes, U, V)
    return y
